# revision 63
# baseline (speedup 1.0000x reference)
"""DilateAttention Trainium2 Bass kernel.

Problem: q,k,v [16, 128, 64, 64] f32; per-pixel attention over 9 dilated
(dil=2) 3x3 neighbors per head (4 heads x 32 dim); out [16, 64, 64, 128].

Sharding: data-parallel over batch B across 8 cores (2 images/core).

Layout: channel-major ([128 ch partitions, pixels free]). K and V are kept
as zero-padded 68x68 bf16 images loaded via gpsimd (software-DGE) casting
DMAs straight from the f32 HBM tensors; the zero padding reproduces torch
Unfold semantics exactly, including the exp(0) softmax denominator terms
at borders.

Per 4-row chunk (256 px), kk grouped by dy (3 groups of 3), software
pipelined 4 chunks deep with stage_b emitted ahead of stage_a so ready
matmuls are never head-of-line blocked:
  products Q*K_kk (DVE bf16 tensor_tensor, last dy group on Pool) ->
  per-head score sums via PE block-ones matmuls into two PSUM windows
  (4+5 planes) -> exp as TWO batched ACT ops (PSUM -> SBUF bf16 at full
  128-partition extent so the per-head -> per-channel broadcast comes out
  of the exp for free) -> softmax denominator DIRECTLY PIXEL-MAJOR via 18
  tiny PE matmuls (stationary = E half-planes [128ch, 128px], moving =
  block-1/32 [128, 4] constant; 4 output rows each, replacing the old
  8x256-row accumulation chain) -> AV products E*V_kk (DVE + Pool tail)
  -> kk-sum accumulated DIRECTLY TRANSPOSED on PE (stationary = P2
  half-planes, moving = identity; lands pixel-major in PSUM, eliminating
  the separate PSUM copy + transpose) -> reciprocal (DVE, [128, 8]) ->
  pixel-major normalize on DVE with a stride-0 head-broadcast AP over the
  per-pixel reciprocals, writing the final f32 -> output DMA per 2 chunks.

Device-verified constraints honored: Pool cannot access PSUM;
scalar_tensor_tensor is DVE-only; compute APs are limited to 3 free dims.
Pool additionally runs the swdge casting-DMA descriptor generation, so
the first 6 chunks keep products entirely on DVE (pool_off) while Pool
generates descriptors; the first K/Q rows bootstrap via HWDGE f32 + cast
so chunk 0 starts ~6us earlier.

Engine busy (cost-model sim, per core): DVE 90.9us, Pool 81.5us, ACT
75.6us, PE 64.7us; wall 104.7us.
"""

import numpy as np
from contextlib import ExitStack

import concourse.bass as bass
import concourse.bacc as bacc
import concourse.tile as tile
from concourse import mybir
from concourse.bass_utils import run_bass_kernel_spmd
from concourse.masks import make_identity

F32 = mybir.dt.float32
BF16 = mybir.dt.bfloat16
MULT = mybir.AluOpType.mult

B, D, H, W = 16, 128, 64, 64
NCORES = 8
BLOC = B // NCORES          # images per core
HEADS, HD = 4, 32
KS, DIL, PAD = 3, 2, 2
HP = H + 2 * PAD            # 68 (y-padded)
WP2 = W + 2 * PAD           # 68 (x-padded)
KK = KS * KS                # 9
SCALE = float(HD) ** -0.5
R = 4                       # image rows per chunk
NC = R * W                  # 256 pixels per chunk
NCHUNK = H // R             # 16
OGRP = 2                    # chunks batched per output DMA
NHALF = NC // 128           # 128-px halves per chunk

# Product planes offloaded to Pool (tail planes of the last dy group).
QK_POOL = 3                 # of the last group's 3 planes
AV_POOL = 1

PROFILE = False


def _bcast_q(qbf, y0):
    """[128, 3, R, W] view of qbf rows y0..y0+R with a 0-step dx axis."""
    base = qbf[:, y0:y0 + R, :]
    return bass.AP(
        tensor=base.tensor,
        offset=base.offset,
        ap=[list(base.ap[0]), [0, KS], [W, R], [1, W]],
    )


def _shift_view(pad_t, y0, idy, i0=0, n=KS):
    """[128, n(idx), R, W] view of padded image at dy=idy for chunk y0,
    idx range [i0, i0+n)."""
    return bass.AP(
        tensor=pad_t.tensor,
        offset=pad_t.offset + (y0 + DIL * idy) * WP2 + DIL * i0,
        ap=[list(pad_t.ap[0]), [DIL, n], [WP2, R], [1, W]],
    )





def _body(ctx: ExitStack, tc: tile.TileContext, out_ap, q_ap, k_ap, v_ap):
    nc = tc.nc

    consts = ctx.enter_context(tc.tile_pool(name="consts", bufs=1))
    perb = ctx.enter_context(tc.tile_pool(name="perb", bufs=2))
    pgp = ctx.enter_context(tc.tile_pool(name="pgp", bufs=4))
    egp = ctx.enter_context(tc.tile_pool(name="egp", bufs=6))
    p2p = ctx.enter_context(tc.tile_pool(name="p2p", bufs=4))
    work = ctx.enter_context(tc.tile_pool(name="work", bufs=6))
    outbuf = ctx.enter_context(tc.tile_pool(name="outbuf", bufs=3))
    # S split 4+5 planes; single-buffered pools alternate with the exps.
    psSA = ctx.enter_context(tc.tile_pool(name="psSA", bufs=1, space="PSUM"))
    psSB = ctx.enter_context(tc.tile_pool(name="psSB", bufs=1, space="PSUM"))
    psZO = ctx.enter_context(tc.tile_pool(name="psZO", bufs=3, space="PSUM"))
    boot = ctx.enter_context(tc.tile_pool(name="boot", bufs=1))

    # Constant stationary matrices.
    blockones = consts.tile([128, 128], BF16)   # 1 if same head
    identb = consts.tile([128, 128], BF16)
    bo32c = consts.tile([128, HEADS], BF16)     # [c, h] = 1/32 if h(c)==h
    nc.vector.memset(blockones, 0.0)
    nc.vector.memset(bo32c, 0.0)
    for h in range(HEADS):
        s = slice(h * HD, (h + 1) * HD)
        nc.vector.memset(blockones[s, s], 1.0)
        nc.vector.memset(bo32c[s, h:h + 1], 1.0 / HD)
    make_identity(nc, identb)

    qf = q_ap.rearrange("b d h w -> b d (h w)")
    out_flat = out_ap.rearrange("b h w d -> b (h w) d")

    def alloc_b(b, memsets=True):
        """Allocate per-image tiles and clear padding borders."""
        qbf = perb.tile([128, H, W], BF16, name="qbf")
        kpad = perb.tile([128, HP, WP2], BF16, name="kpad")
        vpad = perb.tile([128, HP, WP2], BF16, name="vpad")
        if memsets:
            border_memsets((qbf, kpad, vpad))
        return qbf, kpad, vpad

    def border_memsets(tens):
        for t in tens[1:]:
            nc.gpsimd.memset(t[:, 0:PAD, :], 0.0)
            nc.gpsimd.memset(t[:, HP - PAD:HP, :], 0.0)
            nc.gpsimd.memset(t[:, PAD:HP - PAD, 0:PAD], 0.0)
            nc.gpsimd.memset(t[:, PAD:HP - PAD, WP2 - PAD:WP2], 0.0)

    def load_piece(tens, b, which, rows):
        """One tensor's casting swdge DMA (f32 HBM -> bf16 SBUF) for a row
        range. Pieces are emitted at different points in the chunk schedule
        so Pool-engine descriptor generation stays spread."""
        qbf, kpad, vpad = tens
        rs = slice(rows[0], rows[1])
        ps = slice(PAD + rows[0], PAD + rows[1])
        qsrc = qf[b].rearrange("d (h w) -> d h w", w=W)
        if which == "q":
            nc.gpsimd.dma_start(out=qbf[:, rs, :], in_=qsrc[:, rs, :])
        elif which == "k":
            nc.gpsimd.dma_start(out=kpad[:, ps, PAD:WP2 - PAD],
                                in_=k_ap[b][:, rs, :])
        else:
            nc.gpsimd.dma_start(out=vpad[:, ps, PAD:WP2 - PAD],
                                in_=v_ap[b][:, rs, :])

    NSA = 4  # planes in the first S/exp batch (rest in the second)

    def stage_a(tens, ci, pool_off=False):
        """QK products -> per-head score matmuls -> exp (2 batched ops).
        Returns the E tile [128, KK, R, W]. pool_off keeps the products
        off Pool while it chews swdge descriptor generation."""
        qbf, kpad, vpad = tens
        y0 = ci * R
        qv = _bcast_q(qbf, y0)
        P = pgp.tile([128, KK, R, W], BF16, name="P", tag="P")
        qk_pool = 0 if pool_off else QK_POOL
        for g in range(KS):  # g == idy
            nd = KS - qk_pool if g == KS - 1 else KS
            if nd > 0:
                nc.vector.tensor_mul(P[:, g * KS:g * KS + nd], qv[:, 0:nd],
                                     _shift_view(kpad, y0, g, 0, nd))
            if nd < KS:
                nc.gpsimd.tensor_mul(P[:, g * KS + nd:(g + 1) * KS],
                                     qv[:, nd:KS],
                                     _shift_view(kpad, y0, g, nd, KS - nd))
        E = egp.tile([128, KK, R, W], BF16, name="E", tag="E")
        SA = psSA.tile([128, NSA, R, W], F32, name="SA")
        SB = psSB.tile([128, KK - NSA, R, W], F32, name="SB")
        # Bank-aligned 2KB PSUM windows: N=512 matmuls over plane pairs.
        for i in range(0, NSA, 2):
            nc.tensor.matmul(SA[:, i:i + 2], blockones, P[:, i:i + 2],
                             start=True, stop=True)
        for i in range(0, KK - NSA - 1, 2):
            nc.tensor.matmul(SB[:, i:i + 2], blockones,
                             P[:, NSA + i:NSA + i + 2],
                             start=True, stop=True)
        nc.tensor.matmul(SB[:, KK - NSA - 1], blockones, P[:, KK - 1],
                         start=True, stop=True)
        nc.scalar.activation(out=E[:, 0:NSA], in_=SA,
                             func=mybir.ActivationFunctionType.Exp,
                             scale=SCALE)
        nc.scalar.activation(out=E[:, NSA:KK], in_=SB,
                             func=mybir.ActivationFunctionType.Exp,
                             scale=SCALE)
        return E

    state = {"obf": None, "rt": None}

    def stage_b1(tens, b, ci, E, pool_off=False):
        """Pixel-major Z via E-as-stationary matmuls, then AV products."""
        qbf, kpad, vpad = tens
        av_pool = 0 if pool_off else AV_POOL
        y0 = ci * R
        zo = psZO.tile([128, NC + 4 * NHALF], F32, name="zo")
        # Z_t[px, h] for each 128-px half: stationary = E half-plane
        # [128ch, 128px], moving = block-1/32 [128ch, 4h]; 4-row matmuls.
        for j in range(NHALF):
            dst = zo[:, NC + 4 * j:NC + 4 * (j + 1)]
            for kk in range(KK):
                st = E[:, kk, 2 * j:2 * j + 2, :]
                nc.tensor.matmul(dst, st, bo32c,
                                 start=(kk == 0), stop=(kk == KK - 1))
        P2 = p2p.tile([128, KK, R, W], BF16, name="P2", tag="P2")
        for g in range(KS):
            nd = KS - av_pool if g == KS - 1 else KS
            if nd > 0:
                nc.vector.tensor_mul(P2[:, g * KS:g * KS + nd],
                                     E[:, g * KS:g * KS + nd],
                                     _shift_view(vpad, y0, g, 0, nd))
            if nd < KS:
                nc.gpsimd.tensor_mul(P2[:, g * KS + nd:(g + 1) * KS],
                                     E[:, g * KS + nd:(g + 1) * KS],
                                     _shift_view(vpad, y0, g, nd, KS - nd))
        return zo, P2

    def stage_b(tens, b, ci, bstate):
        """Transposed kk-sum straight into PSUM (P2 half-planes as the
        stationary, identity moving), pixel-major normalize, store."""
        zo, P2 = bstate
        if ci % OGRP == 0:
            state["outs"] = outbuf.tile([128, OGRP, NHALF, HEADS, HD], F32,
                                        name="outs")
        outs = state["outs"]
        # O_t[px, c] per 128-px half, accumulated over kk: 128-row matmuls.
        for j in range(NHALF):
            dst = zo[:, j * 128:(j + 1) * 128]
            for kk in range(KK):
                st = P2[:, kk, 2 * j:2 * j + 2, :]
                nc.tensor.matmul(dst, st, identb,
                                 start=(kk == 0), stop=(kk == KK - 1))
        rt = work.tile([128, 4 * NHALF], F32, name="Rt", tag="Rt")
        nc.vector.reciprocal_approx_fast(out=rt,
                                         in_=zo[:, NC:NC + 4 * NHALF])
        # out[px, j, h, d] = O_t[px, j*128 + h*32+d] * rt[px, 4j+h];
        # rt broadcast across d via a 0-step axis. Pool cannot touch PSUM,
        # so this runs on DVE.
        ot_v = bass.AP(
            tensor=zo.tensor, offset=zo.offset,
            ap=[list(zo.ap[0]), [128, NHALF], [HD, HEADS], [1, HD]])
        rt_v = bass.AP(
            tensor=rt.tensor, offset=rt.offset,
            ap=[list(rt.ap[0]), [HEADS, NHALF], [1, HEADS], [0, HD]])
        nc.vector.tensor_mul(outs[:, ci % OGRP], ot_v, rt_v)
        if ci % OGRP == OGRP - 1:
            c0 = ci - (OGRP - 1)
            dst = out_flat[b][c0 * NC:(ci + 1) * NC].rearrange(
                "(o j p) d -> p o j d", p=128, o=OGRP)
            nc.sync.dma_start(out=dst,
                              in_=outs.rearrange("p o j h d -> p o j (h d)"))

    # Software pipeline: stage_b of task i-DEPTH is emitted BEFORE stage_a
    # of task i so ready-to-run O/Z matmuls are never head-of-line blocked
    # behind S matmuls still waiting on products. b=1's load DMAs are
    # spread across the b=0 chunk schedule so Pool-engine swdge descriptor
    # generation never starves the product work.
    DEPTH = 4
    tens = [None] * BLOC
    tens[0] = alloc_b(0, memsets=False)
    # Bootstrap k/q rows 0-15 via HWDGE f32 + ACT cast so the first chunks
    # never wait on Pool-engine swdge descriptor generation.
    qbf0, kpad0, vpad0 = tens[0]
    kboot = boot.tile([128, 16, W], F32, name="kboot")
    qboot = boot.tile([128, 16, W], F32, name="qboot")
    nc.sync.dma_start(out=kboot, in_=k_ap[0][:, 0:16, :])
    nc.sync.dma_start(out=qboot,
                      in_=qf[0].rearrange("d (h w) -> d h w", w=W)[:, 0:16, :])
    nc.vector.tensor_copy(out=kpad0[:, PAD:PAD + 16, PAD:WP2 - PAD],
                          in_=kboot)
    nc.scalar.copy(out=qbf0[:, 0:16, :], in_=qboot)
    border_memsets(tens[0])
    load_piece(tens[0], 0, "k", (16, 32))
    load_piece(tens[0], 0, "q", (16, 32))
    load_piece(tens[0], 0, "v", (0, 32))
    load_piece(tens[0], 0, "k", (32, 64))
    load_piece(tens[0], 0, "q", (32, 64))
    load_piece(tens[0], 0, "v", (32, 64))
    # During the first POOL_OFF_UNTIL emission slots, Pool runs only swdge
    # descriptor generation (b0's tail + all of b1's loads); products stay
    # on DVE so Pool's in-order stream can't stall them.
    POOL_OFF_UNTIL = 6
    tasks = [(b, ci) for b in range(BLOC) for ci in range(NCHUNK)]
    pend = []
    loadmap = {2: "k", 3: "v", 5: "q"}
    for idx, (b, ci) in enumerate(tasks):
        if b == 0 and BLOC > 1 and ci in loadmap:
            if ci == min(loadmap):
                tens[1] = alloc_b(1, memsets=False)
            load_piece(tens[1], 1, loadmap[ci], (0, 64))
            if ci == max(loadmap):
                border_memsets(tens[1])
        if len(pend) >= DEPTH:
            pb, pci, pEg = pend.pop(0)
            off = idx < POOL_OFF_UNTIL
            stage_b(tens[pb], pb, pci,
                    stage_b1(tens[pb], pb, pci, pEg, pool_off=off))
        Eg = stage_a(tens[b], ci, pool_off=idx < POOL_OFF_UNTIL)
        pend.append((b, ci, Eg))
    for pb, pci, pEg in pend:
        stage_b(tens[pb], pb, pci,
                stage_b1(tens[pb], pb, pci, pEg, pool_off=True))


_CACHE = {}


def _build():
    if "nc" not in _CACHE:
        nc = bacc.Bacc("TRN2", target_bir_lowering=False, debug=False,
                       num_devices=NCORES)
        q = nc.dram_tensor("q", [BLOC, D, H, W], F32, kind="ExternalInput").ap()
        k = nc.dram_tensor("k", [BLOC, D, H, W], F32, kind="ExternalInput").ap()
        v = nc.dram_tensor("v", [BLOC, D, H, W], F32, kind="ExternalInput").ap()
        out = nc.dram_tensor("out", [BLOC, H, W, D], F32,
                             kind="ExternalOutput").ap()
        with tile.TileContext(nc) as tc:
            with ExitStack() as ctx:
                _body(ctx, tc, out, q, k, v)
        nc.compile()
        _CACHE["nc"] = nc
    return _CACHE["nc"]


def kernel(q, k, v):
    q = np.ascontiguousarray(np.asarray(q), dtype=np.float32)
    k = np.ascontiguousarray(np.asarray(k), dtype=np.float32)
    v = np.ascontiguousarray(np.asarray(v), dtype=np.float32)
    nc = _build()
    in_maps = [
        {
            "q": np.ascontiguousarray(q[i * BLOC:(i + 1) * BLOC]),
            "k": np.ascontiguousarray(k[i * BLOC:(i + 1) * BLOC]),
            "v": np.ascontiguousarray(v[i * BLOC:(i + 1) * BLOC]),
        }
        for i in range(NCORES)
    ]
    res = run_bass_kernel_spmd(nc, in_maps, list(range(NCORES)),
                               trace=PROFILE)
    out = np.concatenate([r["out"] for r in res.results], axis=0)
    if PROFILE:
        kernel.last_exec_time_ns = res.exec_time_ns
        kernel.last_results = res
    return out


if __name__ == "__main__":
    nc = _build()
    print("build OK")
    from concourse.timeline_sim import TimelineSim
    tl = TimelineSim(nc, trace=False)
    t = tl.simulate()
    print(f"TimelineSim: {t/1000.0:.1f} us")
